# revision 7
# baseline (speedup 1.0000x reference)
"""Trainium2 Bass kernel for nn_BinaryLinear (8-core SPMD).

Computes: z = x @ binarize(w).T + binarize(b); out = relu((z - mean)/(std + eps))
with binarize(t) = (t > mean(t)) per-tensor; row-wise layernorm over out_features.

Strategy (v2):
  - Data-parallel over the 8192-token batch: each core computes 1024 token rows.
  - Weight binarize+transpose is SPLIT across cores (512 rows each) against the
    global mean (tiny AllReduce of partial sums); the transposed binary weight
    is distributed by 4 pipelined AllGathers (1MB/rank each, staged by k-range).
  - All transposes run on the PE (128x128 transpose-mode + DVE PSUM eviction):
    the DMA XBAR transpose path serializes against copy DMAs and measured
    ~10us per tile; PE does it in ~0.5us.
  - Matmul in bf16: binarized weights are exact in bf16; only x's bf16 cast
    quantizes (~1.6e-3 norm rel err).
  - b_q enters PSUM via a K=1 ones-matmul opening each accumulation group.
  - Layernorm stats ride the PSUM evictions (bn_stats/bn_aggr); normalize+relu
    is a fused scalar-engine activation; z round-trips DRAM in f32.
  - Two HWDGE queues used deliberately: sync (SP) carries the weight-side and
    main-loop traffic; scalar (ACT) carries the x-side and normalize traffic,
    so AllReduce-gated weight loads can't head-block the x pipeline.
"""
import numpy as np

import concourse.bass as bass
import concourse.mybir as mybir
import concourse.tile as tile
from concourse import bacc
from concourse.bass_utils import run_bass_kernel_spmd
from concourse.masks import make_identity

N_CORES = 8
T_FULL = 8192
D_IN = 4096
D_OUT = 4096
T_SHARD = T_FULL // N_CORES    # 1024
O_SHARD = D_OUT // N_CORES     # 512
P = 128
NK = D_IN // P                 # 32 k-tiles
NM = T_SHARD // P              # 8 token tiles
NJ = D_OUT // O_SHARD          # 8 o-blocks
NG = 4                         # AllGather stages (k-ranges of 8 k-tiles)
KPG = NK // NG                 # 8 k-tiles per stage
NWC = O_SHARD // P             # 4 weight o-chunks per core
HD = D_IN // 2                 # half width for f32 streaming
EPS = 1e-5
F32 = mybir.dt.float32
BF16 = mybir.dt.bfloat16
FP8 = mybir.dt.float8e4

_cache: dict = {}
last_exec_time_ns = None


def _maybe_patch_ldw_opt():
    """Optional experiment: let walrus hoist LDWEIGHTS (default args disable it)."""
    import os
    if os.environ.get("BASS_LDW_OPT", "") != "1":
        return
    import concourse.bass_utils as bu
    if getattr(bu, "_ldw_patched", False):
        return
    orig = bu.run_command

    def patched(argv, **kw):
        argv = ["--enable-ldw-opt=true" if a == "--enable-ldw-opt=false" else a
                for a in argv]
        return orig(argv, **kw)

    bu.run_command = patched
    bu._ldw_patched = True


def _bcast_ap(handle_ap, n_part):
    """Stride-0 partition-broadcast AP for a scalar DRAM location."""
    return bass.AP(
        tensor=handle_ap.tensor, offset=handle_ap.offset,
        ap=[[0, n_part], [1, 1]],
    )


def _build():
    nc = bacc.Bacc("TRN2", target_bir_lowering=False, debug=False,
                   num_devices=N_CORES)
    x_in = nc.dram_tensor("x", [T_SHARD, D_IN], F32, kind="ExternalInput")
    w_in = nc.dram_tensor("w", [O_SHARD, D_IN], F32, kind="ExternalInput")
    b_in = nc.dram_tensor("b", [D_OUT], F32, kind="ExternalInput")
    out_ext = nc.dram_tensor("out", [T_SHARD, D_OUT], F32, kind="ExternalOutput")

    with tile.TileContext(nc) as tc:
        with (
            tc.tile_pool(name="xT_pool", bufs=1) as xT_pool,
            tc.tile_pool(name="wshare", bufs=8) as wshare,
            tc.tile_pool(name="f32w", bufs=2) as f32w,
            tc.tile_pool(name="f32x", bufs=2) as f32x,
            tc.tile_pool(name="xbf_pool", bufs=2) as xbf_pool,
            tc.tile_pool(name="zev_pool", bufs=4) as zev_pool,
            tc.tile_pool(name="wtr_pool", bufs=2) as wtr_pool,
            tc.tile_pool(name="small", bufs=1) as small,
            tc.tile_pool(name="psum", bufs=4, space="PSUM") as psum,
            tc.tile_pool(name="psum_tr", bufs=3, space="PSUM") as psum_tr,
            tc.tile_pool(name="psum_s", bufs=1, space="PSUM") as psum_s,
            tc.tile_pool(name="dram", bufs=1, space="DRAM") as dram,
        ):
            identity = small.tile([P, P], BF16)
            make_identity(nc, identity)

            # ---- A: partial sums of w slice (sync queue) -> AllReduce ----
            colsums = small.tile([P, NWC * 2], F32)
            for c4 in range(NWC):
                for h in range(2):
                    wh = f32w.tile([P, HD], F32, name=f"ws{c4}_{h}", tag="f32w")
                    nc.sync.dma_start(
                        out=wh[:],
                        in_=w_in[c4 * P:(c4 + 1) * P, h * HD:(h + 1) * HD])
                    nc.vector.reduce_sum(colsums[:, c4 * 2 + h:c4 * 2 + h + 1],
                                         wh[:], axis=mybir.AxisListType.X)
            rowsum = small.tile([P, 1], F32)
            nc.vector.reduce_sum(rowsum[:], colsums[:], axis=mybir.AxisListType.X)
            ones_f32 = small.tile([P, 1], F32)
            nc.vector.memset(ones_f32[:], 1.0)
            psum_tot = psum_s.tile([1, 1], F32)
            nc.tensor.matmul(psum_tot[:], rowsum[:], ones_f32[:],
                             start=True, stop=True)
            ar_sb = small.tile([1, 8], F32)
            nc.vector.memset(ar_sb[:], 0.0)
            nc.vector.tensor_copy(out=ar_sb[:, 0:1], in_=psum_tot[:])
            ar_in = dram.tile([8], F32)
            ar_out = dram.tile([8], F32, addr_space="Shared")
            nc.gpsimd.dma_start(out=ar_in[:].rearrange("(o d) -> o d", o=1),
                                in_=ar_sb[:])
            nc.gpsimd.collective_compute(
                "AllReduce", mybir.AluOpType.add,
                replica_groups=[list(range(N_CORES))],
                ins=[ar_in.opt()], outs=[ar_out.opt()],
            )
            thr_sb = small.tile([P, 1], F32)
            nc.gpsimd.dma_start(out=thr_sb[:], in_=_bcast_ap(ar_out.opt(), P))
            nc.vector.tensor_scalar_mul(thr_sb[:], thr_sb[:],
                                        1.0 / (D_OUT * D_IN))

            # ---- B: bias binarize (scalar queue, local) ----
            b_sb = f32x.tile([1, D_OUT], F32, name="b_sb", tag="f32x")
            nc.scalar.dma_start(out=b_sb[:],
                                in_=b_in[:].rearrange("(o d) -> o d", o=1))
            b_sum = small.tile([1, 1], F32)
            nc.vector.reduce_sum(b_sum[:], b_sb[:], axis=mybir.AxisListType.X)
            b_mean = small.tile([1, 1], F32)
            nc.vector.tensor_scalar_mul(b_mean[:], b_sum[:], 1.0 / D_OUT)
            b_q = small.tile([1, D_OUT], BF16)
            nc.vector.tensor_scalar(
                out=b_q[:], in0=b_sb[:], scalar1=b_mean[:], scalar2=None,
                op0=mybir.AluOpType.is_gt,
            )
            ones_bf = small.tile([1, P], BF16)
            nc.vector.memset(ones_bf[:], 1.0)

            # ---- C: binarize w slice -> bf16 resident; PE-transpose; AGs ----
            wq_res = [wshare.tile([P, D_IN], BF16, name=f"wq{c4}", tag="wsh")
                      for c4 in range(NWC)]
            for c4 in range(NWC):
                for h in range(2):
                    wh = f32w.tile([P, HD], F32, name=f"wb{c4}_{h}", tag="f32w")
                    nc.sync.dma_start(
                        out=wh[:],
                        in_=w_in[c4 * P:(c4 + 1) * P, h * HD:(h + 1) * HD])
                    nc.vector.tensor_scalar(
                        out=wq_res[c4][:, h * HD:(h + 1) * HD], in0=wh[:],
                        scalar1=thr_sb[:], scalar2=None,
                        op0=mybir.AluOpType.is_gt,
                    )
            w_qT_own = [dram.tile([KPG * P, O_SHARD], FP8, name=f"wqT_own{g}")
                        for g in range(NG)]
            w_qT_all = [dram.tile([N_CORES, KPG * P, O_SHARD], FP8,
                                  name=f"wqT_all{g}", addr_space="Shared")
                        for g in range(NG)]
            for g in range(NG):
                for kk in range(KPG):
                    k = g * KPG + kk
                    wtr = wtr_pool.tile([P, O_SHARD], FP8, name=f"wtr{k}",
                                        tag="wtr")
                    for c4 in range(NWC):
                        pt = psum_tr.tile([P, P], BF16, name=f"wpt{k}_{c4}",
                                          tag="ptr")
                        nc.tensor.transpose(pt[:],
                                            wq_res[c4][:, k * P:(k + 1) * P],
                                            identity[:])
                        nc.vector.tensor_copy(out=wtr[:, c4 * P:(c4 + 1) * P],
                                              in_=pt[:])
                    nc.sync.dma_start(out=w_qT_own[g][kk * P:(kk + 1) * P, :],
                                      in_=wtr[:])
                nc.gpsimd.collective_compute(
                    "AllGather", mybir.AluOpType.bypass,
                    replica_groups=[list(range(N_CORES))],
                    ins=[w_qT_own[g].opt()], outs=[w_qT_all[g].opt()],
                )

            # ---- D: x load/cast (scalar queue) + PE-transpose into xT ----
            xT = []
            for k in range(NK):
                t = xT_pool.tile([P, T_SHARD], BF16, name=f"xT{k}", tag=f"xT{k}")
                xT.append(t)
            for mx in range(NM):
                for h in range(2):
                    xh = f32x.tile([P, HD], F32, name=f"xh{mx}_{h}", tag="f32x")
                    nc.scalar.dma_start(
                        out=xh[:],
                        in_=x_in[mx * P:(mx + 1) * P, h * HD:(h + 1) * HD])
                    xbf = xbf_pool.tile([P, HD], BF16, name=f"xbf{mx}_{h}",
                                        tag="xbf")
                    nc.scalar.copy(out=xbf[:], in_=xh[:])
                    for kk in range(HD // P):
                        k = h * (HD // P) + kk
                        pt = psum_tr.tile([P, P], BF16, name=f"xpt{mx}_{k}",
                                          tag="ptr")
                        nc.tensor.transpose(pt[:], xbf[:, kk * P:(kk + 1) * P],
                                            identity[:])
                        nc.vector.tensor_copy(
                            out=xT[k][:, mx * P:(mx + 1) * P], in_=pt[:])

            # ---- E: main loop: matmul + stats + fused normalize ----
            z_dram = [dram.tile([P, D_OUT], F32, name=f"z{m}") for m in range(NM)]
            stats = [small.tile([P, NJ, 6], F32, name=f"stats{m}")
                     for m in range(NM)]
            for j in range(NJ):
                wg = []
                for g in range(NG):
                    t = wshare.tile([P, KPG, O_SHARD], FP8, name=f"wg{j}_{g}",
                                    tag="wsh")
                    nc.sync.dma_start(
                        out=t[:],
                        in_=w_qT_all[g][j].rearrange("(kk p) o -> p kk o", p=P))
                    wg.append(t)
                for m in range(NM):
                    ps = psum.tile([P, O_SHARD], F32, name=f"ps{j}_{m}", tag="ps")
                    nc.tensor.matmul(ps[:], ones_bf[:],
                                     b_q[:, j * O_SHARD:(j + 1) * O_SHARD],
                                     start=True, stop=False)
                    for g in range(NG):
                        for kk in range(KPG):
                            k = g * KPG + kk
                            nc.tensor.matmul(
                                ps[:], xT[k][:, m * P:(m + 1) * P],
                                wg[g][:, kk, :],
                                start=False, stop=(k == NK - 1))
                    zev = zev_pool.tile([P, O_SHARD], F32, name=f"zev{j}_{m}",
                                        tag="zev")
                    nc.vector.tensor_copy(out=zev[:], in_=ps[:])
                    nc.vector.bn_stats(out=stats[m][:, j, :], in_=zev[:])
                    nc.sync.dma_start(
                        out=z_dram[m][:, j * O_SHARD:(j + 1) * O_SHARD],
                        in_=zev[:])
                    if j == NJ - 1:
                        mv = small.tile([P, 2], F32, name=f"mv{m}")
                        nc.vector.bn_aggr(out=mv[:], in_=stats[m][:])
                        std = small.tile([P, 1], F32, name=f"std{m}")
                        nc.scalar.sqrt(std[:], mv[:, 1:2])
                        nc.vector.tensor_scalar_add(std[:], std[:], EPS)
                        rstd = small.tile([P, 1], F32, name=f"rstd{m}")
                        nc.vector.reciprocal(rstd[:], std[:])
                        shift = small.tile([P, 1], F32, name=f"shift{m}")
                        nc.vector.tensor_mul(shift[:], mv[:, 0:1], rstd[:])
                        nc.vector.tensor_scalar_mul(shift[:], shift[:], -1.0)
                        for h in range(2):
                            nin = f32w.tile([P, HD], F32, name=f"nin{m}_{h}",
                                            tag="f32w")
                            nc.scalar.dma_start(
                                out=nin[:],
                                in_=z_dram[m][:, h * HD:(h + 1) * HD])
                            for q in range(HD // O_SHARD):
                                zq = zev_pool.tile([P, O_SHARD], F32,
                                                   name=f"zq{m}_{h}_{q}",
                                                   tag="zev")
                                nc.scalar.activation(
                                    out=zq[:],
                                    in_=nin[:, q * O_SHARD:(q + 1) * O_SHARD],
                                    func=mybir.ActivationFunctionType.Relu,
                                    bias=shift[:], scale=rstd[:],
                                )
                                nc.scalar.dma_start(
                                    out=out_ext[
                                        m * P:(m + 1) * P,
                                        h * HD + q * O_SHARD:
                                        h * HD + (q + 1) * O_SHARD],
                                    in_=zq[:])

    nc.finalize()
    return nc


def kernel(x: np.ndarray, weight: np.ndarray, b: np.ndarray) -> np.ndarray:
    global last_exec_time_ns
    import os
    x = np.ascontiguousarray(x, dtype=np.float32)
    weight = np.ascontiguousarray(weight, dtype=np.float32)
    b = np.ascontiguousarray(b, dtype=np.float32)
    assert x.shape == (T_FULL, D_IN) and weight.shape == (D_OUT, D_IN)

    if "nc" not in _cache:
        _maybe_patch_ldw_opt()
        _cache["nc"] = _build()
    nc = _cache["nc"]

    in_maps = [
        {
            "x": x[c * T_SHARD:(c + 1) * T_SHARD],
            "w": weight[c * O_SHARD:(c + 1) * O_SHARD],
            "b": b,
        }
        for c in range(N_CORES)
    ]
    trace = os.environ.get("BASS_KERNEL_TRACE", "") == "1"
    res = run_bass_kernel_spmd(nc, in_maps, list(range(N_CORES)), trace=trace)
    last_exec_time_ns = res.exec_time_ns
    return np.concatenate([res.results[c]["out"] for c in range(N_CORES)],
                          axis=0)
